# revision 2
# baseline (speedup 1.0000x reference)
"""NT-Xent (SimCLR) contrastive loss on 8 Trainium2 NeuronCores.

Strategy (data-parallel over rows of the 8192x8192 similarity matrix):
  reps = concat(emb_i, emb_j)                     # [8192, 256]
  Each core c gets reps cyclically rolled by -c*1024 rows, so its own
  1024 rows are always local rows 0..1023 -> one identical SPMD program.
  On device (per core):
    - normalize all 8192 rows (z = u / ||u||), cast bf16
    - transpose to z^T [256, 8192] via PE transposes (matmul layout)
    - sim row-block [1024, 8192] = z_own^T.T @ z^T in [128,512] psum tiles
    - exp(2*sim) + row-sum on the scalar engine (accum_out)
    - positive-pair diag extracted from psum via identity-mask reduce
  Host: denom = rowsum - e^2 (self-sim of unit rows), loss = mean(2*pos - log denom).
"""

import sys
import numpy as np

sys.path.insert(0, "/opt/trn_rl_repo")

B = 4096
D = 256
N2 = 2 * B          # 8192 rows of reps
NCORES = 8
RPC = N2 // NCORES  # 1024 rows per core
NCHUNK = 16         # column chunks of 512
CHW = 512           # chunk width
GRP = 3             # chunks per psum group (1536 wide, 3 banks)
NGRP = 6            # ceil(16/3): widths 1536*5 + 512
TEMP = 0.5
SCALE = 1.0 / TEMP  # 2.0

_CACHE = {}


def _build(repeat=1):
    """Build the SPMD Bass program once; returns (nc,).

    repeat>1 emits the whole body R times back-to-back in one NEFF — used
    only for wall-clock differencing (axon round-trip is ~100ms, so a
    single 0.1ms kernel is unmeasurable without on-device repetition).
    """
    import concourse.bass as bass
    import concourse.tile as tile
    from concourse import bacc, mybir
    from concourse.masks import make_identity

    f32 = mybir.dt.float32
    bf16 = mybir.dt.bfloat16
    Alu = mybir.AluOpType
    Act = mybir.ActivationFunctionType

    from concourse.hw_specs import get_activation_tables

    class _PinnedBacc(bacc.Bacc):
        """Pin ACT-table selection to natural_log_exp_and_others (holds
        Ln+Exp+Copy+Square+Identity) so the kernel needs one table load
        instead of thrashing between exp-only and ln-only tables."""

        def insert_act_table_loads(self):
            import bass_rust as _bass_rust
            from concourse import mybir as _mb

            has_activation = any(
                isinstance(i, _mb.InstActivation)
                for b in self.main_func.blocks
                for i in b.instructions
            )
            if not has_activation:
                return
            tables = [
                (name, funcs if name == "natural_log_exp_and_others" else set())
                for name, funcs in get_activation_tables(self.m.arch).items()
            ]
            _bass_rust.insert_act_table_loads(self, tables)

    nc = _PinnedBacc(
        "TRN2", target_bir_lowering=False, debug=False, num_devices=NCORES
    )

    reps_d = nc.dram_tensor("reps", [N2, D], f32, kind="ExternalInput").ap()
    rowsums_d = nc.dram_tensor(
        "rowsums", [128, 8 * NGRP], f32, kind="ExternalOutput"
    ).ap()
    pos_d = nc.dram_tensor("pos", [128, 8], f32, kind="ExternalOutput").ap()

    with tile.TileContext(nc) as tc:
        from contextlib import ExitStack

        with ExitStack() as ctx:
            const_pool = ctx.enter_context(tc.tile_pool(name="const", bufs=1))
            ident_bf = const_pool.tile([128, 128], bf16)
            ident_f32 = const_pool.tile([128, 128], f32)
            make_identity(nc, ident_bf[:])
            make_identity(nc, ident_f32[:])

            u_pool = ctx.enter_context(tc.tile_pool(name="u", bufs=8))
            sq_pool = ctx.enter_context(tc.tile_pool(name="sq", bufs=2))
            ss_pool = ctx.enter_context(tc.tile_pool(name="ss", bufs=4))
            z_pool = ctx.enter_context(tc.tile_pool(name="z", bufs=4))
            ptr_pool = ctx.enter_context(
                tc.tile_pool(name="ptr", bufs=2, space="PSUM")
            )
            rt_pool = ctx.enter_context(tc.tile_pool(name="rt", bufs=32))
            psb_pool = ctx.enter_context(
                tc.tile_pool(name="psb", bufs=2, space="PSUM")
            )
            exp_pool = ctx.enter_context(tc.tile_pool(name="expo", bufs=2))
            scr_pool = ctx.enter_context(tc.tile_pool(name="scr", bufs=2))
            out_pool = ctx.enter_context(tc.tile_pool(name="outp", bufs=2))

            for _rep in range(repeat):
              rowsums = out_pool.tile([128, 8 * NGRP], f32, tag="rs", name="rowsums")
              pos = out_pool.tile([128, 8], f32, tag="pos", name="pos")

              # ---------------- Phase A: build z^T [2][128, 8192] bf16 ----------
              repsT = [[None] * NCHUNK, [None] * NCHUNK]
              for n in range(NCHUNK):
                  ss = ss_pool.tile([128, 4], f32, tag="ss")
                  inv = ss_pool.tile([128, 4], f32, tag="inv")
                  lns = ss_pool.tile([128, 4], f32, tag="lns")
                  us = []
                  for tl in range(4):
                      t = 4 * n + tl
                      u = u_pool.tile([128, D], f32)
                      nc.sync.dma_start(u[:], reps_d[t * 128 : (t + 1) * 128, :])
                      us.append(u)
                      sq = sq_pool.tile([128, D], f32)
                      nc.vector.scalar_tensor_tensor(
                          out=sq[:],
                          in0=u[:],
                          scalar=1.0,
                          in1=u[:],
                          op0=Alu.bypass,
                          op1=Alu.mult,
                          accum_out=ss[:, tl : tl + 1],
                      )
                  # inv_norm = exp(-0.5 * ln(sumsq)); ln/exp share one ACT table
                  nc.scalar.activation(lns[:], ss[:], Act.Ln)
                  nc.scalar.activation(inv[:], lns[:], Act.Exp, scale=-0.5)
                  ptrs = [
                      ptr_pool.tile([128, CHW], bf16, tag="ptr", name=f"ptr{k}")
                      for k in range(2)
                  ]
                  for tl in range(4):
                      z = z_pool.tile([128, D], bf16)
                      nc.vector.tensor_scalar_mul(z[:], us[tl][:], inv[:, tl : tl + 1])
                      for k in range(2):
                          nc.tensor.transpose(
                              ptrs[k][:, tl * 128 : (tl + 1) * 128],
                              z[:, k * 128 : (k + 1) * 128],
                              ident_bf[:],
                          )
                  for k in range(2):
                      rt = rt_pool.tile([128, CHW], bf16, tag="rt")
                      nc.vector.tensor_copy(rt[:], ptrs[k][:])
                      repsT[k][n] = rt

              # ---------------- Phase B: sim row-block, exp, rowsum -------------
              for g in range(NGRP):
                  chunks = list(range(GRP * g, min(GRP * (g + 1), NCHUNK)))
                  w = CHW * len(chunks)
                  for m in range(8):
                      ps = psb_pool.tile([128, w], f32, tag="psb")
                      for k in range(2):
                          lhsT = repsT[k][m // 4][:, (m % 4) * 128 : (m % 4 + 1) * 128]
                          for ci, n in enumerate(chunks):
                              nc.tensor.matmul(
                                  ps[:, ci * CHW : (ci + 1) * CHW],
                                  lhsT,
                                  repsT[k][n][:],
                                  start=(k == 0),
                                  stop=(k == 1),
                                  skip_group_check=True,
                              )
                      # positive-pair diag: local col 4096 + m*128 + p
                      pc = 4096 + m * 128
                      if pc // CHW in chunks:
                          off = pc - chunks[0] * CHW
                          scr = scr_pool.tile([128, 128], f32, tag="scr")
                          nc.vector.scalar_tensor_tensor(
                              out=scr[:],
                              in0=ps[:, off : off + 128],
                              scalar=1.0,
                              in1=ident_f32[:],
                              op0=Alu.bypass,
                              op1=Alu.mult,
                              accum_out=pos[:, m : m + 1],
                          )
                      ex = exp_pool.tile([128, w], bf16, tag="expo")
                      nc.scalar.activation(
                          ex[:],
                          ps[:],
                          Act.Exp,
                          scale=SCALE,
                          accum_out=rowsums[:, m * NGRP + g : m * NGRP + g + 1],
                      )

              nc.sync.dma_start(rowsums_d[:], rowsums[:])
              nc.sync.dma_start(pos_d[:], pos[:])

    nc.compile()
    return nc


def _get_nc(repeat=1):
    key = ("nc", repeat)
    if key not in _CACHE:
        _CACHE[key] = _build(repeat)
    return _CACHE[key]


def _make_in_maps(emb_i: np.ndarray, emb_j: np.ndarray) -> list:
    reps = np.concatenate(
        [np.asarray(emb_i, np.float32), np.asarray(emb_j, np.float32)], axis=0
    )
    return [{"reps": np.roll(reps, -c * RPC, axis=0)} for c in range(NCORES)]


def kernel(emb_i: np.ndarray, emb_j: np.ndarray) -> np.ndarray:
    from concourse.bass_utils import run_bass_kernel_spmd

    nc = _get_nc()
    in_maps = _make_in_maps(emb_i, emb_j)
    res = run_bass_kernel_spmd(nc, in_maps, core_ids=list(range(NCORES)))
    return _combine(res.results)


def _combine(results) -> np.ndarray:
    # per core: rowsums [128, 8*NGRP] ([p, m*NGRP+g]), pos [128, 8] ([p, m])
    S = np.empty((NCORES, 8, 128), np.float64)   # [c, m, p] row sums
    P = np.empty((NCORES, 8, 128), np.float64)
    for c in range(NCORES):
        rs = np.asarray(results[c]["rowsums"], np.float64)  # [128, 48]
        S[c] = rs.reshape(128, 8, NGRP).sum(axis=2).T
        P[c] = np.asarray(results[c]["pos"], np.float64).T
    denom = S - np.exp(2.0)  # subtract self-similarity exp(1/T)
    loss = (2.0 * P - np.log(denom)).mean()
    return np.float32(loss)



# revision 10
# speedup vs baseline: 1.4069x; 1.4069x over previous
"""NT-Xent (SimCLR) contrastive loss on 8 Trainium2 NeuronCores.

Symmetric data-parallel strategy over the 8192x8192 similarity matrix:
  reps = concat(emb_i, emb_j)                      # [8192, 256]
  sim is symmetric, so each unordered 1024x1024 block pair is computed
  once.  Core c gets reps rolled by -c*1024 and keeps the first 5120
  rows; it computes its own row block (local rows 0..1023, global block
  c) against local column blocks b = 0..4 (global blocks c..c+4):
    - d = 1..3 block pairs are unique to one core; exp row-sums cover
      the row block, and column-sums (by symmetry) cover the partner
      block's denominators.
    - d = 0 (diagonal) contributes row-sums only.
    - d = 4 is computed by both endpoint cores; each uses row-sums only.
  On device (per core):
    - normalize 5120 rows (z = u / ||u||), cast bf16, PE-transpose to
      z^T [2][128, 5120]
    - per (b, m): sim strip [128, 1024] in PSUM, exp(2*sim) on ScalarE
      with accum_out row-sums; E bf16 kept for b in {1,2,3}
    - column sums of E via ones-matmul accumulation chains on the PE
    - positive-pair diag extracted from the b=4 PSUM via identity-mask
      reduce on DVE
  Host: assemble denominators from row/col sums, subtract e^2 self-sim,
  loss = mean(2*pos - log denom).
"""

import sys
import numpy as np

sys.path.insert(0, "/opt/trn_rl_repo")

B = 4096
D = 256
N2 = 2 * B          # 8192 rows of reps
NCORES = 8
RPC = N2 // NCORES  # 1024 rows per core
NBLK = 5            # column blocks per core (symmetric coverage)
LROWS = NBLK * RPC  # 5120 local rows needed per core
NT = LROWS // 128   # 40 u-tiles
TEMP = 0.5
SCALE = 1.0 / TEMP  # 2.0

_CACHE = {}


def _build(repeat=1):
    """Build the SPMD Bass program once; returns nc."""
    import concourse.bass as bass
    import concourse.tile as tile
    from concourse import bacc, mybir
    from concourse.masks import make_identity

    f32 = mybir.dt.float32
    bf16 = mybir.dt.bfloat16
    Alu = mybir.AluOpType
    Act = mybir.ActivationFunctionType

    from concourse.hw_specs import get_activation_tables

    class _PinnedBacc(bacc.Bacc):
        """Pin ACT-table selection to natural_log_exp_and_others (holds
        Ln+Exp+Copy+Square+Identity) so the kernel needs one table load
        instead of thrashing between exp-only and ln-only tables."""

        def insert_act_table_loads(self):
            import bass_rust as _bass_rust

            has_activation = any(
                isinstance(i, mybir.InstActivation)
                for b in self.main_func.blocks
                for i in b.instructions
            )
            if not has_activation:
                return
            tables = [
                (name, funcs if name == "natural_log_exp_and_others" else set())
                for name, funcs in get_activation_tables(self.m.arch).items()
            ]
            _bass_rust.insert_act_table_loads(self, tables)

    nc = _PinnedBacc(
        "TRN2", target_bir_lowering=False, debug=False, num_devices=NCORES
    )

    reps_d = nc.dram_tensor("reps", [LROWS, D], f32, kind="ExternalInput").ap()
    rowsums_d = nc.dram_tensor(
        "rowsums", [128, NBLK * 8], f32, kind="ExternalOutput"
    ).ap()
    pos_d = nc.dram_tensor("pos", [128, 8], f32, kind="ExternalOutput").ap()
    colsums_d = nc.dram_tensor(
        "colsums", [128, 3 * 512], f32, kind="ExternalOutput"
    ).ap()

    with tile.TileContext(nc) as tc:
        from contextlib import ExitStack

        with ExitStack() as ctx:
            const_pool = ctx.enter_context(tc.tile_pool(name="const", bufs=1))
            ident_bf = const_pool.tile([128, 128], bf16)
            ident_f32 = const_pool.tile([128, 128], f32)
            ones_bf = const_pool.tile([128, 64], bf16)
            make_identity(nc, ident_bf[:])
            make_identity(nc, ident_f32[:])
            nc.vector.memset(ones_bf[:], 1.0)

            u_pool = ctx.enter_context(tc.tile_pool(name="u", bufs=10))
            sq_pool = ctx.enter_context(tc.tile_pool(name="sq", bufs=2))
            ss_pool = ctx.enter_context(tc.tile_pool(name="ss", bufs=4))
            z_pool = ctx.enter_context(tc.tile_pool(name="z", bufs=4))
            # PSUM budget (8 banks): psb 2x[128,1024]f32 (4) +
            # ptr 2x[128,1024]bf16 (2) + cs 2x[128,512]f32 (2)
            ptr_pool = ctx.enter_context(
                tc.tile_pool(name="ptr", bufs=2, space="PSUM")
            )
            rt_pool = ctx.enter_context(tc.tile_pool(name="rt", bufs=10))
            psb_pool = ctx.enter_context(
                tc.tile_pool(name="psb", bufs=2, space="PSUM")
            )
            cs_pool = ctx.enter_context(
                tc.tile_pool(name="cs", bufs=2, space="PSUM")
            )
            e_pool = ctx.enter_context(tc.tile_pool(name="ep", bufs=10))
            scr_pool = ctx.enter_context(tc.tile_pool(name="scr", bufs=2))
            esc_pool = ctx.enter_context(tc.tile_pool(name="esc", bufs=2))
            out_pool = ctx.enter_context(tc.tile_pool(name="outp", bufs=2))

            for _rep in range(repeat):
              rowsums = out_pool.tile(
                  [128, NBLK * 8], f32, tag="rs", name="rowsums"
              )
              pos = out_pool.tile([128, 8], f32, tag="pos", name="pos")
              colsb = out_pool.tile(
                  [128, 3 * 512], f32, tag="cb", name="colsb"
              )

              # ------- Phase A: normalize + transpose -> zT [2][128, 5120] ----
              repsT = [[None] * NBLK, [None] * NBLK]
              for b in range(NBLK):
                  ss = ss_pool.tile([128, 8], f32, tag="ss")
                  inv = ss_pool.tile([128, 8], f32, tag="inv")
                  lns = ss_pool.tile([128, 8], f32, tag="lns")
                  us = []
                  for t in range(8):
                      g = 8 * b + t
                      u = u_pool.tile([128, D], f32)
                      nc.sync.dma_start(u[:], reps_d[g * 128 : (g + 1) * 128, :])
                      us.append(u)
                      sq = sq_pool.tile([128, D], f32)
                      nc.vector.scalar_tensor_tensor(
                          out=sq[:],
                          in0=u[:],
                          scalar=1.0,
                          in1=u[:],
                          op0=Alu.bypass,
                          op1=Alu.mult,
                          accum_out=ss[:, t : t + 1],
                      )
                  # inv_norm = exp(-0.5 * ln(sumsq)); ln/exp share one table
                  nc.scalar.activation(lns[:], ss[:], Act.Ln)
                  nc.scalar.activation(inv[:], lns[:], Act.Exp, scale=-0.5)
                  ptrs = [
                      ptr_pool.tile([128, 1024], bf16, tag="ptr", name=f"ptr{k}")
                      for k in range(2)
                  ]
                  for t in range(8):
                      z = z_pool.tile([128, D], bf16)
                      nc.vector.tensor_scalar_mul(z[:], us[t][:], inv[:, t : t + 1])
                      for k in range(2):
                          nc.tensor.transpose(
                              ptrs[k][:, t * 128 : (t + 1) * 128],
                              z[:, k * 128 : (k + 1) * 128],
                              ident_bf[:],
                          )
                  for k in range(2):
                      rt = rt_pool.tile([128, 1024], bf16, tag="rt")
                      nc.vector.tensor_copy(rt[:], ptrs[k][:])
                      repsT[k][b] = rt

              # ------- Phase B: sim strips, exp+rowsum, colsums, pos ----------
              for b in range(NBLK):
                  es = []  # E tiles of this block (kept for colsum if 1<=b<=3)
                  for m in range(8):
                      ps = psb_pool.tile([128, 1024], f32, tag="psb")
                      for k in range(2):
                          lhsT = repsT[k][0][:, m * 128 : (m + 1) * 128]
                          for h in range(2):
                              nc.tensor.matmul(
                                  ps[:, h * 512 : (h + 1) * 512],
                                  lhsT,
                                  repsT[k][b][:, h * 512 : (h + 1) * 512],
                                  start=(k == 0),
                                  stop=(k == 1),
                                  skip_group_check=True,
                              )
                      if b == 4:
                          # positive-pair diag: local col 4096 + m*128 + p
                          scr = scr_pool.tile([128, 128], f32, tag="scr")
                          nc.vector.scalar_tensor_tensor(
                              out=scr[:],
                              in0=ps[:, m * 128 : (m + 1) * 128],
                              scalar=1.0,
                              in1=ident_f32[:],
                              op0=Alu.bypass,
                              op1=Alu.mult,
                              accum_out=pos[:, m : m + 1],
                          )
                      if 1 <= b <= 3:
                          ex = e_pool.tile([128, 1024], bf16, tag="ep")
                      else:
                          ex = esc_pool.tile([128, 1024], bf16, tag="esc")
                      nc.scalar.activation(
                          ex[:],
                          ps[:],
                          Act.Exp,
                          scale=SCALE,
                          accum_out=rowsums[:, b * 8 + m : b * 8 + m + 1],
                      )
                      es.append(ex)
                  if 1 <= b <= 3:
                      # column sums by symmetry: cs[h-slot, j] for partner rows
                      cs = cs_pool.tile([128, 512], f32, tag="cs")
                      for h in range(2):
                          for m in range(8):
                              nc.tensor.matmul(
                                  cs[64 * h : 64 * h + 64, :],
                                  ones_bf[:],
                                  es[m][:, h * 512 : (h + 1) * 512],
                                  start=(m == 0),
                                  stop=(m == 7),
                                  skip_group_check=True,
                              )
                      nc.vector.tensor_copy(
                          colsb[:, (b - 1) * 512 : b * 512], cs[:]
                      )

              nc.sync.dma_start(rowsums_d[:], rowsums[:])
              nc.sync.dma_start(pos_d[:], pos[:])
              nc.sync.dma_start(colsums_d[:], colsb[:])

    nc.compile()
    return nc


def _get_nc(repeat=1):
    key = ("nc", repeat)
    if key not in _CACHE:
        _CACHE[key] = _build(repeat)
    return _CACHE[key]


def _make_in_maps(emb_i: np.ndarray, emb_j: np.ndarray) -> list:
    reps = np.concatenate(
        [np.asarray(emb_i, np.float32), np.asarray(emb_j, np.float32)], axis=0
    )
    rolled = np.concatenate([reps, reps[: LROWS - RPC]], axis=0)
    return [
        {"reps": np.ascontiguousarray(rolled[c * RPC : c * RPC + LROWS])}
        for c in range(NCORES)
    ]


def kernel(emb_i: np.ndarray, emb_j: np.ndarray) -> np.ndarray:
    from concourse.bass_utils import run_bass_kernel_spmd

    nc = _get_nc()
    in_maps = _make_in_maps(emb_i, emb_j)
    res = run_bass_kernel_spmd(nc, in_maps, core_ids=list(range(NCORES)))
    return _combine(res.results)


def _combine(results) -> np.ndarray:
    # Per core: rowsums [128, 5*8] (col b*8+m), pos [128, 8] (col m),
    # colsums [128, 3*512]: partition 0 = cols 0..511 of block b (at col
    # range (b-1)*512), partition 64 = cols 512..1023.
    denom = np.zeros((NCORES, RPC), np.float64)  # [block q, offset j]
    pos = np.empty((NCORES, RPC), np.float64)
    for c in range(NCORES):
        rs = np.asarray(results[c]["rowsums"], np.float64)  # [128, 40]
        # local row = 128*m + p -> offset j in block c
        s = rs.reshape(128, NBLK, 8).sum(axis=1)  # [p, m]
        denom[c] += s.T.reshape(RPC)
        p = np.asarray(results[c]["pos"], np.float64)  # [128, 8]
        pos[c] = p.T.reshape(RPC)
        cs = np.asarray(results[c]["colsums"], np.float64)  # [128, 1536]
        for b in range(1, 4):
            col = np.concatenate(
                [cs[0, (b - 1) * 512 : b * 512], cs[64, (b - 1) * 512 : b * 512]]
            )  # [1024] cols j of local block b = global block (c+b)%8
            denom[(c + b) % NCORES] += col
    denom -= np.exp(SCALE)  # subtract self-similarity exp(1/T)
    loss = (SCALE * pos - np.log(denom)).mean()
    return np.float32(loss)


# revision 20
# speedup vs baseline: 1.4815x; 1.0530x over previous
"""NT-Xent (SimCLR) contrastive loss on 8 Trainium2 NeuronCores.

Symmetric data-parallel strategy over the 8192x8192 similarity matrix:
  reps = concat(emb_i, emb_j)                      # [8192, 256]
  sim is symmetric, so each unordered 1024x1024 block pair is computed
  once.  Core c gets reps rolled by -c*1024 and keeps the first 5120
  rows; it computes its own row block (local rows 0..1023, global block
  c) against local column blocks b = 0..4 (global blocks c..c+4):
    - d = 1..3 block pairs are unique to one core; exp row-sums cover
      the row block, and column-sums (by symmetry) cover the partner
      block's denominators.
    - d = 0 (diagonal) contributes row-sums only.
    - d = 4 is computed by both endpoint cores; each uses row-sums only.
  On device (per core):
    - normalize 5120 rows (z = u / ||u||), cast bf16, PE-transpose to
      z^T [2][128, 5120]
    - per (b, m): sim strip [128, 1024] in PSUM, exp(2*sim) on ScalarE
      with accum_out row-sums; E bf16 kept for b in {1,2,3}
    - column sums of E via ones-matmul accumulation chains on the PE
    - positive-pair diag extracted from the b=4 PSUM via identity-mask
      reduce on DVE
  Host: assemble denominators from row/col sums, subtract e^2 self-sim,
  loss = mean(2*pos - log denom).
"""

import sys
import numpy as np

sys.path.insert(0, "/opt/trn_rl_repo")

B = 4096
D = 256
N2 = 2 * B          # 8192 rows of reps
NCORES = 8
RPC = N2 // NCORES  # 1024 rows per core
NBLK = 5            # column blocks per core (symmetric coverage)
LROWS = NBLK * RPC  # 5120 local rows needed per core
NT = LROWS // 128   # 40 u-tiles
TEMP = 0.5
SCALE = 1.0 / TEMP  # 2.0

_CACHE = {}


def _build(repeat=1):
    """Build the SPMD Bass program once; returns nc."""
    import concourse.bass as bass
    import concourse.tile as tile
    from concourse import bacc, mybir
    from concourse.masks import make_identity

    f32 = mybir.dt.float32
    bf16 = mybir.dt.bfloat16
    f8 = mybir.dt.float8e4
    Alu = mybir.AluOpType
    Act = mybir.ActivationFunctionType
    DR = mybir.MatmulPerfMode.DoubleRow

    from concourse.hw_specs import get_activation_tables

    class _PinnedBacc(bacc.Bacc):
        """Pin ACT-table selection to natural_log_exp_and_others (holds
        Ln+Exp+Copy+Square+Identity) so the kernel needs one table load
        instead of thrashing between exp-only and ln-only tables."""

        def insert_act_table_loads(self):
            import bass_rust as _bass_rust

            has_activation = any(
                isinstance(i, mybir.InstActivation)
                for b in self.main_func.blocks
                for i in b.instructions
            )
            if not has_activation:
                return
            tables = [
                (name, funcs if name == "natural_log_exp_and_others" else set())
                for name, funcs in get_activation_tables(self.m.arch).items()
            ]
            _bass_rust.insert_act_table_loads(self, tables)

    nc = _PinnedBacc(
        "TRN2", target_bir_lowering=False, debug=False, num_devices=NCORES
    )

    reps_d = nc.dram_tensor("reps", [LROWS, D], f32, kind="ExternalInput").ap()
    rowsums_d = nc.dram_tensor(
        "rowsums", [128, NBLK * 8], f32, kind="ExternalOutput"
    ).ap()
    pos_d = nc.dram_tensor("pos", [128, 8], f32, kind="ExternalOutput").ap()
    colsums_d = nc.dram_tensor(
        "colsums", [128, 3 * 512], f32, kind="ExternalOutput"
    ).ap()

    with tile.TileContext(nc) as tc:
        from contextlib import ExitStack

        with ExitStack() as ctx:
            const_pool = ctx.enter_context(tc.tile_pool(name="const", bufs=1))
            ident_bf = const_pool.tile([128, 128], bf16)
            ident_f32 = const_pool.tile([128, 128], f32)
            ones_bf = const_pool.tile([128, 64], bf16)
            make_identity(nc, ident_bf[:])
            make_identity(nc, ident_f32[:])
            nc.vector.memset(ones_bf[:], 1.0)

            u_pool = ctx.enter_context(tc.tile_pool(name="u", bufs=10))
            sq_pool = ctx.enter_context(tc.tile_pool(name="sq", bufs=2))
            ss_pool = ctx.enter_context(tc.tile_pool(name="ss", bufs=4))
            z_pool = ctx.enter_context(tc.tile_pool(name="z", bufs=4))
            # PSUM budget (8 banks): psb 2x[128,1024]f32 (4) +
            # ptr 2x[128,1024]bf16 (2) + cs 2x[128,512]f32 (2)
            ptr_pool = ctx.enter_context(
                tc.tile_pool(name="ptr", bufs=2, space="PSUM")
            )
            rt_pool = ctx.enter_context(tc.tile_pool(name="rt", bufs=10))
            psb_pool = ctx.enter_context(
                tc.tile_pool(name="psb", bufs=2, space="PSUM")
            )
            cs_pool = ctx.enter_context(
                tc.tile_pool(name="cs", bufs=2, space="PSUM")
            )
            e_pool = ctx.enter_context(tc.tile_pool(name="ep", bufs=10))
            scr_pool = ctx.enter_context(tc.tile_pool(name="scr", bufs=2))
            esc_pool = ctx.enter_context(tc.tile_pool(name="esc", bufs=2))
            out_pool = ctx.enter_context(tc.tile_pool(name="outp", bufs=2))

            for _rep in range(repeat):
              rowsums = out_pool.tile(
                  [128, NBLK * 8], f32, tag="rs", name="rowsums"
              )
              pos = out_pool.tile([128, 8], f32, tag="pos", name="pos")
              colsb = out_pool.tile(
                  [128, 3 * 512], f32, tag="cb", name="colsb"
              )

              # ------- Phase A: normalize + transpose -> zT [128, 2, 5120] ---
              repsT = [None] * NBLK
              for b in range(NBLK):
                  ss = ss_pool.tile([128, 8], f32, tag="ss")
                  inv = ss_pool.tile([128, 8], f32, tag="inv")
                  lns = ss_pool.tile([128, 8], f32, tag="lns")
                  us = []
                  for t in range(8):
                      g = 8 * b + t
                      u = u_pool.tile([128, D], f32)
                      nc.sync.dma_start(u[:], reps_d[g * 128 : (g + 1) * 128, :])
                      us.append(u)
                      sq = sq_pool.tile([128, D], f32)
                      nc.vector.scalar_tensor_tensor(
                          out=sq[:],
                          in0=u[:],
                          scalar=1.0,
                          in1=u[:],
                          op0=Alu.bypass,
                          op1=Alu.mult,
                          accum_out=ss[:, t : t + 1],
                      )
                  # inv_norm = 16 * exp(-0.5 * ln(sumsq)): the x16 pre-scale
                  # moves z into fp8e4's normal range; exp scale divides by
                  # 256 to compensate.  ln/exp share one ACT table.
                  nc.scalar.activation(lns[:], ss[:], Act.Ln, scale=1.0 / 256.0)
                  nc.scalar.activation(inv[:], lns[:], Act.Exp, scale=-0.5)
                  ptrs = [
                      ptr_pool.tile([128, 1024], bf16, tag="ptr", name=f"ptr{k}")
                      for k in range(2)
                  ]
                  for t in range(8):
                      z = z_pool.tile([128, D], bf16)
                      nc.vector.tensor_scalar_mul(z[:], us[t][:], inv[:, t : t + 1])
                      for k in range(2):
                          nc.tensor.transpose(
                              ptrs[k][:, t * 128 : (t + 1) * 128],
                              z[:, k * 128 : (k + 1) * 128],
                              ident_bf[:],
                          )
                  rt = rt_pool.tile([128, 2, 1024], f8, tag="rt")
                  for k in range(2):
                      nc.vector.tensor_copy(rt[:, k, :], ptrs[k][:])
                  repsT[b] = rt

              # ------- Phase B: sim strips, exp+rowsum, colsums, pos ----------
              for b in range(NBLK):
                  es = []  # E tiles of this block (kept for colsum if 1<=b<=3)
                  for m in range(8):
                      ps = psb_pool.tile([128, 1024], f32, tag="psb")
                      lhsT = repsT[0][:, :, m * 128 : (m + 1) * 128]
                      for h in range(2):
                          nc.tensor.matmul(
                              ps[:, h * 512 : (h + 1) * 512],
                              lhsT,
                              repsT[b][:, :, h * 512 : (h + 1) * 512],
                              start=True,
                              stop=True,
                              perf_mode=DR,
                              skip_group_check=True,
                          )
                      if b == 4:
                          # positive-pair diag: local col 4096 + m*128 + p
                          scr = scr_pool.tile([128, 128], f32, tag="scr")
                          nc.vector.scalar_tensor_tensor(
                              out=scr[:],
                              in0=ps[:, m * 128 : (m + 1) * 128],
                              scalar=1.0,
                              in1=ident_f32[:],
                              op0=Alu.bypass,
                              op1=Alu.mult,
                              accum_out=pos[:, m : m + 1],
                          )
                      if 1 <= b <= 3:
                          ex = e_pool.tile([128, 1024], bf16, tag="ep")
                      else:
                          ex = esc_pool.tile([128, 1024], bf16, tag="esc")
                      nc.scalar.activation(
                          ex[:],
                          ps[:],
                          Act.Exp,
                          scale=SCALE / 256.0,
                          accum_out=rowsums[:, b * 8 + m : b * 8 + m + 1],
                      )
                      es.append(ex)
                  if 1 <= b <= 3:
                      # column sums by symmetry: cs[h-slot, j] for partner rows
                      cs = cs_pool.tile([128, 512], f32, tag="cs")
                      for h in range(2):
                          for m in range(8):
                              nc.tensor.matmul(
                                  cs[64 * h : 64 * h + 64, :],
                                  ones_bf[:],
                                  es[m][:, h * 512 : (h + 1) * 512],
                                  start=(m == 0),
                                  stop=(m == 7),
                                  skip_group_check=True,
                              )
                      nc.vector.tensor_copy(
                          colsb[:, (b - 1) * 512 : b * 512], cs[:]
                      )

              nc.sync.dma_start(rowsums_d[:], rowsums[:])
              nc.sync.dma_start(pos_d[:], pos[:])
              nc.sync.dma_start(colsums_d[:], colsb[:])

    nc.compile()
    return nc


def _get_nc(repeat=1):
    key = ("nc", repeat)
    if key not in _CACHE:
        _CACHE[key] = _build(repeat)
    return _CACHE[key]


def _make_in_maps(emb_i: np.ndarray, emb_j: np.ndarray) -> list:
    reps = np.concatenate(
        [np.asarray(emb_i, np.float32), np.asarray(emb_j, np.float32)], axis=0
    )
    rolled = np.concatenate([reps, reps[: LROWS - RPC]], axis=0)
    return [
        {"reps": np.ascontiguousarray(rolled[c * RPC : c * RPC + LROWS])}
        for c in range(NCORES)
    ]


def kernel(emb_i: np.ndarray, emb_j: np.ndarray) -> np.ndarray:
    from concourse.bass_utils import run_bass_kernel_spmd

    nc = _get_nc()
    in_maps = _make_in_maps(emb_i, emb_j)
    res = run_bass_kernel_spmd(nc, in_maps, core_ids=list(range(NCORES)))
    return _combine(res.results)


def _combine(results) -> np.ndarray:
    # Per core: rowsums [128, 5*8] (col b*8+m), pos [128, 8] (col m),
    # colsums [128, 3*512]: partition 0 = cols 0..511 of block b (at col
    # range (b-1)*512), partition 64 = cols 512..1023.
    denom = np.zeros((NCORES, RPC), np.float64)  # [block q, offset j]
    pos = np.empty((NCORES, RPC), np.float64)
    for c in range(NCORES):
        rs = np.asarray(results[c]["rowsums"], np.float64)  # [128, 40]
        # local row = 128*m + p -> offset j in block c
        s = rs.reshape(128, NBLK, 8).sum(axis=1)  # [p, m]
        denom[c] += s.T.reshape(RPC)
        p = np.asarray(results[c]["pos"], np.float64)  # [128, 8], sim * 256
        pos[c] = p.T.reshape(RPC) / 256.0
        cs = np.asarray(results[c]["colsums"], np.float64)  # [128, 1536]
        for b in range(1, 4):
            col = np.concatenate(
                [cs[0, (b - 1) * 512 : b * 512], cs[64, (b - 1) * 512 : b * 512]]
            )  # [1024] cols j of local block b = global block (c+b)%8
            denom[(c + b) % NCORES] += col
    denom -= np.exp(SCALE)  # subtract self-similarity exp(1/T)
    loss = (SCALE * pos - np.log(denom)).mean()
    return np.float32(loss)
